# revision 1
# baseline (speedup 1.0000x reference)
"""MixHop layer (gnn_message_passing) as a Trainium2 Bass/Tile SPMD kernel.

Math reformulation (A = sparse adjacency with edge weights, row=dst, col=src):
    x0 = x @ W0 + b0
    x1 = A @ (x @ W1 + b1) = (A @ x) @ W1 + d1 (x) b1      d1 = A @ 1
    x2 = A @ A @ (x @ W2 + b2) = (A @ A @ x) @ W2 + d2 (x) b2,  d2 = A @ d1
so only two sparse propagations of the raw features are needed:
    y1 = A @ x   (pass A),   y2 = A @ y1  (pass B, after all-gather of y1)

Sharding: destination rows are split across 8 cores (12544 rows each, padded
from 100000 to 100352). Edges are partitioned by destination shard and sorted
into 128-row destination blocks; block edge lists are split by source chunk
(dma_gather indices are int16, so sources are gathered from 4 chunks of
25088 rows) and padded to multiples of 128 ("edge tiles"). For each edge
tile the kernel:
  - gathers the 128 source rows (512B each) via dma_gather (one call per
    (stage, chunk), a stage being a group of consecutive dest blocks),
  - builds a [128 edges x 128 rows] selection matrix sel[e, r] =
    w[e] * (row_local[e] == r) with one DVE tensor_scalar op,
  - accumulates psum[rows, feat] += sel.T @ gathered on the tensor engine.
Per-(block, chunk) tile counts are padded to the max across cores so a
single SPMD program serves all 8 cores.
"""

import os
import sys

import numpy as np

for _p in ("/opt/trn_rl_repo",):
    if os.path.isdir(_p) and _p not in sys.path:
        sys.path.insert(0, _p)

import concourse.bacc as bacc
import concourse.bass as bass
import concourse.mybir as mybir
import concourse.tile as tile
from concourse.bass_utils import run_bass_kernel_spmd

F32 = mybir.dt.float32
I16 = mybir.dt.int16

N_CORES = 8
P = 128          # partitions / rows per block / edges per tile
NCHUNK = 4       # source chunks (int16 index reach)
STAGE_TILE_CAP = 80  # max edge tiles staged in SBUF at once


# ---------------------------------------------------------------------------
# host-side preprocessing
# ---------------------------------------------------------------------------

def _prep(x, row, col, edge_weight, n_cores=N_CORES):
    N, C = x.shape
    E = row.shape[0]
    S = -(-N // (n_cores * P)) * P          # shard rows per core
    NP = S * n_cores                        # padded node count
    B = S // P                              # dest blocks per core
    CH = NP // NCHUNK                       # source-chunk rows
    assert CH % P == 0 and CH <= 32768

    # weighted degrees (biases of hop1/hop2 after the reformulation)
    w64 = edge_weight.astype(np.float64)
    d1 = np.bincount(row, weights=w64, minlength=NP)[:NP]
    d2 = np.bincount(row, weights=w64 * d1[col], minlength=NP)[:NP]
    d1 = d1.astype(np.float32)
    d2 = d2.astype(np.float32)

    # sort edges by (destination block, source chunk)
    gblk = (row // P).astype(np.int64)
    chunk = (col // CH).astype(np.int64)
    order = np.lexsort((chunk, gblk))
    gblk_s = gblk[order]
    chunk_s = chunk[order]
    col_s = col[order].astype(np.int64)
    w_s = edge_weight[order].astype(np.float32)
    rloc_s = (row[order] % P).astype(np.float32)

    nblk = NP // P
    grp = gblk_s * NCHUNK + chunk_s                     # sorted group key
    cnt = np.bincount(grp, minlength=nblk * NCHUNK)     # edges per (gblk, c)
    tiles = -(-cnt // P)                                # ceil
    # uniform per-(local block, chunk) tile count across cores
    T_BC = tiles.reshape(n_cores, B, NCHUNK).max(axis=0).astype(np.int64)
    empty = T_BC.sum(axis=1) == 0
    T_BC[empty, 0] = 1                                  # >=1 tile per block
    T_B = T_BC.sum(axis=1)
    LT = int(T_B.sum())                                 # edge tiles/core/pass

    stages = _make_stages(T_B)

    # tile-order base of every (block, chunk): stage -> chunk -> block
    base_bc = np.zeros((B, NCHUNK), dtype=np.int64)
    pos0 = 0
    call_lens = []                                      # per stage: 4 lens
    for (b0, nb, _) in stages:
        lens = []
        for c in range(NCHUNK):
            n = 0
            for b in range(b0, b0 + nb):
                base_bc[b, c] = pos0
                pos0 += T_BC[b, c]
                n += T_BC[b, c]
            lens.append(int(n))
        call_lens.append(lens)
    assert pos0 == LT

    # final position of every edge
    grp_start = np.zeros(nblk * NCHUNK + 1, dtype=np.int64)
    np.cumsum(cnt, out=grp_start[1:])
    rank = np.arange(E, dtype=np.int64) - grp_start[grp]
    b_local = gblk_s % B
    e_core = gblk_s // B
    pos = P * base_bc[b_local, chunk_s] + rank

    L = P * LT
    eidx = np.zeros((n_cores, L), dtype=np.int16)       # pad -> chunk row 0
    erow = np.zeros((n_cores, L), dtype=np.float32)
    ew = np.zeros((n_cores, L), dtype=np.float32)       # pad -> weight 0
    eidx[e_core, pos] = (col_s - chunk_s * CH).astype(np.int16)
    erow[e_core, pos] = rloc_s
    ew[e_core, pos] = w_s

    # dma_gather index wrapping: idx i -> [i % 16, i // 16]; call bases are
    # multiples of 128 so a global wrap equals per-call wraps. Replicated to
    # all 8 Q7 core groups (16-partition stripes).
    eidx16 = np.ascontiguousarray(
        np.tile(eidx.reshape(n_cores, L // 16, 16).transpose(0, 2, 1),
                (1, 8, 1)))                             # [cores, 128, L//16]
    # edge j -> partition j%128, tile j//128
    erow = np.ascontiguousarray(erow.reshape(n_cores, LT, P).transpose(0, 2, 1))
    ew = np.ascontiguousarray(ew.reshape(n_cores, LT, P).transpose(0, 2, 1))

    x_pad = np.zeros((NP, C), dtype=np.float32)
    x_pad[:N] = x
    xT = np.ascontiguousarray(
        x_pad.reshape(n_cores, S, C).transpose(0, 2, 1))   # [cores, C, S]

    d1_sb = np.ascontiguousarray(
        d1.reshape(n_cores, B, P).transpose(0, 2, 1))      # [cores, 128, B]
    d2_sb = np.ascontiguousarray(
        d2.reshape(n_cores, B, P).transpose(0, 2, 1))

    return dict(N=N, C=C, S=S, NP=NP, B=B, CH=CH, T_BC=T_BC, T_B=T_B, LT=LT,
                stages=stages, call_lens=call_lens, base_bc=base_bc,
                eidx16=eidx16, erow=erow, ew=ew, x_pad=x_pad, xT=xT,
                d1=d1_sb, d2=d2_sb)


def _make_stages(T_B, cap=STAGE_TILE_CAP):
    """Group consecutive blocks into stages of <= cap tiles.
    Returns (first_block, n_blocks, stage_tile_offset) tuples."""
    stages = []
    b = 0
    off = 0
    B = len(T_B)
    while b < B:
        start = b
        soff = off
        tot = 0
        while b < B and (b == start or tot + T_B[b] <= cap):
            tot += int(T_B[b])
            off += int(T_B[b])
            b += 1
        stages.append((start, b - start, soff))
    return stages


# ---------------------------------------------------------------------------
# device program
# ---------------------------------------------------------------------------

def build_program(meta, n_cores=N_CORES, mode="full"):
    N, C, S, NP, B = meta["N"], meta["C"], meta["S"], meta["NP"], meta["B"]
    CH, T_BC, T_B, LT = meta["CH"], meta["T_BC"], meta["T_B"], meta["LT"]
    stages, call_lens, base_bc = (meta["stages"], meta["call_lens"],
                                  meta["base_bc"])
    ts_max = max(int(T_B[b0:b0 + nb].sum()) for b0, nb, _ in stages)

    nc = bacc.Bacc("TRN2", target_bir_lowering=False, debug=False,
                   num_devices=n_cores, num_swdge_queues=4)

    x_full = nc.dram_tensor("x_full", [NP, C], F32, kind="ExternalInput")
    xT_d = nc.dram_tensor("xT", [C, S], F32, kind="ExternalInput")
    wmat_d = nc.dram_tensor("wmat", [C, 3 * C], F32, kind="ExternalInput")
    consts_d = nc.dram_tensor("consts", [P, 5 * P + 2 * B], F32,
                              kind="ExternalInput")
    eidx_d = nc.dram_tensor("eidx", [P, (P * LT) // 16], I16,
                            kind="ExternalInput")
    erow_d = nc.dram_tensor("erow", [P, LT], F32, kind="ExternalInput")
    ew_d = nc.dram_tensor("ew", [P, LT], F32, kind="ExternalInput")
    out_d = nc.dram_tensor("out", [S, 3 * C], F32, kind="ExternalOutput")

    with tile.TileContext(nc) as tc:
        with (
            tc.tile_pool(name="dram", bufs=1, space="DRAM") as dram,
            tc.tile_pool(name="cpool", bufs=1) as cpool,
            tc.tile_pool(name="fpool", bufs=3) as fpool,
            tc.tile_pool(name="mpool", bufs=3) as mpool,
            tc.tile_pool(name="spool", bufs=8) as spool,
            tc.tile_pool(name="vpool", bufs=3) as vpool,
            tc.tile_pool(name="ypsum", bufs=3, space="PSUM") as ypsum,
            tc.tile_pool(name="tpsum", bufs=2, space="PSUM") as tpsum,
            tc.tile_pool(name="xpsum", bufs=3, space="PSUM") as xpsum,
        ):
            y1s = dram.tile([S, C], F32)                       # AG input
            y1f = dram.tile([NP, C], F32, addr_space="Shared")  # AG output

            # resident constants
            consts_t = cpool.tile([P, 5 * P + 2 * B], F32, tag="consts")
            nc.sync.dma_start(consts_t[:], consts_d[:])
            iota_t = consts_t[:, 0 * P:1 * P]
            eye_t = consts_t[:, 1 * P:2 * P]
            b0b_t = consts_t[:, 2 * P:3 * P]
            b1b_t = consts_t[:, 3 * P:4 * P]
            b2b_t = consts_t[:, 4 * P:5 * P]
            d1_t = consts_t[:, 5 * P:5 * P + B]
            d2_t = consts_t[:, 5 * P + B:5 * P + 2 * B]
            wmat_t = cpool.tile([C, 3 * C], F32, tag="wmat")
            nc.sync.dma_start(wmat_t[:], wmat_d[:])
            f_dummy = None
            if mode in ("compute_only", "scatter_only"):
                f_dummy = cpool.tile([P, ts_max, C], F32, tag="fdummy")
                nc.vector.memset(f_dummy[:], 0.0)
            w0_t = wmat_t[:, 0 * C:1 * C]
            w1_t = wmat_t[:, 1 * C:2 * C]
            w2_t = wmat_t[:, 2 * C:3 * C]

            NBMAX = max(nb for _, nb, _ in stages)

            def emit_pass(src_t, w_t, bias_bcast_t, d_col_t, out_col0,
                          writeback, tscatter=False):
                for si, (b0, nb, soff) in enumerate(stages):
                    ts = int(T_B[b0:b0 + nb].sum())
                    # stage metadata loads
                    idx_t = mpool.tile([P, (P * ts_max) // 16], I16, tag="idx")
                    row_t = mpool.tile([P, ts_max], F32, tag="row")
                    w_e_t = mpool.tile([P, ts_max], F32, tag="we")
                    i16o = (P * soff) // 16
                    i16n = (P * ts) // 16
                    nc.sync.dma_start(idx_t[:, :i16n],
                                      eidx_d[:, i16o:i16o + i16n])
                    nc.sync.dma_start(row_t[:, :ts], erow_d[:, soff:soff + ts])
                    nc.sync.dma_start(w_e_t[:, :ts], ew_d[:, soff:soff + ts])
                    # gathers: one per source chunk
                    f_t = (f_dummy if mode in ("compute_only", "scatter_only")
                           else fpool.tile([P, ts_max, C], F32, tag="f"))
                    rel = 0
                    for c in range(0 if mode in ("compute_only", "scatter_only") else NCHUNK):
                        tsc = call_lens[si][c]
                        if tsc == 0:
                            continue
                        nidx = P * tsc
                        nc.gpsimd.dma_gather(
                            out_ap=f_t[:, rel:rel + tsc, :],
                            in_ap=x_full[c * CH:(c + 1) * CH, :]
                            if src_t is None else src_t[c * CH:(c + 1) * CH, :],
                            idxs_ap=idx_t[:, (P * rel) // 16:
                                          (P * rel) // 16 + nidx // 16],
                            num_idxs=nidx,
                            num_idxs_reg=nidx,
                            elem_size=C,
                            single_packet=False,
                            queue_num=c,
                        )
                        rel += tsc
                    if mode == "gather_only":
                        continue
                    # per-stage staging tiles (batched writes/loads)
                    nco = C
                    x_st = (None if writeback
                            else vpool.tile([P, NBMAX, nco], F32, tag="xst"))
                    y_st = (None if tscatter
                            else vpool.tile([P, NBMAX, C], F32, tag="yst"))
                    # per-block scatter matmuls + transforms
                    for b in range(b0, b0 + nb):
                        kk = b - b0
                        y_ps = ypsum.tile([P, C], F32, tag="ypsum")
                        tl = [(int(base_bc[b, c] - soff), int(T_BC[b, c]))
                              for c in range(NCHUNK) if T_BC[b, c] > 0]
                        ntile = sum(n for _, n in tl)
                        k = 0
                        for (g0, n) in tl:
                            for t in range(g0, g0 + n):
                                sel = spool.tile([P, P], F32, tag="sel")
                                nc.vector.tensor_scalar(
                                    out=sel[:],
                                    in0=iota_t,
                                    scalar1=row_t[:, t:t + 1],
                                    scalar2=w_e_t[:, t:t + 1],
                                    op0=mybir.AluOpType.is_equal,
                                    op1=mybir.AluOpType.mult,
                                )
                                if tscatter:
                                    # accumulate y^T directly: [c, rows]
                                    nc.tensor.matmul(
                                        out=y_ps[:],
                                        lhsT=f_t[:, t, :],
                                        rhs=sel[:],
                                        start=(k == 0),
                                        stop=(k == ntile - 1),
                                    )
                                else:
                                    nc.tensor.matmul(
                                        out=y_ps[:],
                                        lhsT=sel[:],
                                        rhs=f_t[:, t, :],
                                        start=(k == 0),
                                        stop=(k == ntile - 1),
                                    )
                                k += 1
                        if mode == "scatter_only":
                            continue

                        if tscatter:
                            yT_sb = vpool.tile([P, C], F32, tag="ytsb")
                            nc.vector.tensor_copy(yT_sb[:], y_ps[:])
                            x_ps = xpsum.tile([P, C], F32, tag="xpsum")
                            nc.tensor.matmul(out=x_ps[:], lhsT=yT_sb[:],
                                             rhs=w_t, start=True, stop=True)
                            tmp = vpool.tile([P, C], F32, tag="tmp")
                            nc.vector.tensor_scalar(
                                out=tmp[:], in0=bias_bcast_t,
                                scalar1=d_col_t[:, b:b + 1], scalar2=None,
                                op0=mybir.AluOpType.mult,
                            )
                            nc.vector.tensor_tensor(
                                out=x_st[:, kk, 0:C], in0=x_ps[:],
                                in1=tmp[:], op=mybir.AluOpType.add)
                        else:
                            # scatter pass: only evacuate y for writeback
                            nc.vector.tensor_copy(y_st[:, kk, :], y_ps[:])

                    if mode == "scatter_only":
                        continue
                    # batched per-stage stores
                    if writeback:
                        nc.sync.dma_start(
                            y1s[b0 * P:(b0 + nb) * P, :]
                            .rearrange("(g p) c -> p g c", p=P),
                            y_st[:, :nb, :])
                    else:
                        nc.sync.dma_start(
                            out_d[b0 * P:(b0 + nb) * P,
                                  out_col0:out_col0 + nco]
                            .rearrange("(g p) c -> p g c", p=P),
                            x_st[:, :nb, :])

            def emit_x0_loop(gsz=8):
                # x0 = x @ W0 + b0: dense pipelined loop, independent of the
                # scatter pipeline (overlaps the AllGather).
                for g0 in range(0, B, gsz):
                    gn = min(gsz, B - g0)
                    xT_t = vpool.tile([C, gsz * P], F32, tag="xT")
                    nc.sync.dma_start(xT_t[:, :gn * P],
                                      xT_d[:, g0 * P:(g0 + gn) * P])
                    x0_st = vpool.tile([P, gsz, C], F32, tag="x0st")
                    for k in range(gn):
                        x0_ps = xpsum.tile([P, C], F32, tag="xpsum")
                        nc.tensor.matmul(
                            out=x0_ps[:], lhsT=xT_t[:, k * P:(k + 1) * P],
                            rhs=w0_t, start=True, stop=True)
                        nc.vector.tensor_tensor(
                            out=x0_st[:, k, :], in0=x0_ps[:], in1=b0b_t,
                            op=mybir.AluOpType.add)
                    nc.sync.dma_start(
                        out_d[g0 * P:(g0 + gn) * P, 0:C]
                        .rearrange("(g p) c -> p g c", p=P),
                        x0_st[:, :gn, :])

            def emit_x1_loop(gsz=8):
                # x1 = y1 @ W1 + d1 (x) b1: dense pipelined transform loop
                # reading y1 back from DRAM (overlaps the AllGather).
                for g0 in range(0, B, gsz):
                    gn = min(gsz, B - g0)
                    y_ld = vpool.tile([P, gsz, C], F32, tag="yld")
                    nc.sync.dma_start(
                        y_ld[:, :gn, :],
                        y1s[g0 * P:(g0 + gn) * P, :]
                        .rearrange("(g p) c -> p g c", p=P))
                    x1_st = vpool.tile([P, gsz, C], F32, tag="x1st")
                    for k in range(gn):
                        b = g0 + k
                        yT_ps = tpsum.tile([P, C], F32, tag="tpsum")
                        nc.tensor.transpose(yT_ps[:], y_ld[:, k, :], eye_t)
                        yT_sb = vpool.tile([P, C], F32, tag="ytsb")
                        nc.vector.tensor_copy(yT_sb[:], yT_ps[:])
                        x_ps = xpsum.tile([P, C], F32, tag="xpsum")
                        nc.tensor.matmul(out=x_ps[:], lhsT=yT_sb[:],
                                         rhs=w1_t, start=True, stop=True)
                        tmp = vpool.tile([P, C], F32, tag="tmp")
                        nc.vector.tensor_scalar(
                            out=tmp[:], in0=b1b_t,
                            scalar1=d1_t[:, b:b + 1], scalar2=None,
                            op0=mybir.AluOpType.mult,
                        )
                        nc.vector.tensor_tensor(
                            out=x1_st[:, k, :], in0=x_ps[:],
                            in1=tmp[:], op=mybir.AluOpType.add)
                    nc.sync.dma_start(
                        out_d[g0 * P:(g0 + gn) * P, C:2 * C]
                        .rearrange("(g p) c -> p g c", p=P),
                        x1_st[:, :gn, :])

            # pass A: y1 = A @ x (scatter + writeback only)
            emit_pass(None, w1_t, b1b_t, d1_t, 0, True)

            if mode not in ("scatter_only", "full_noag"):
                nc.gpsimd.collective_compute(
                    "AllGather",
                    mybir.AluOpType.bypass,
                    replica_groups=[list(range(n_cores))],
                    ins=[y1s[:].opt()],
                    outs=[y1f[:].opt()],
                )

            if mode not in ("gather_only", "scatter_only"):
                emit_x0_loop()
                emit_x1_loop()

            # pass B: y2 = A @ y1, x2 = y2 @ W2 + d2 (x) b2
            # (transposed scatter accumulates y2^T; no transpose needed)
            emit_pass(y1f, w2_t, b2b_t, d2_t, 2 * C, False, tscatter=True)

    nc.compile()
    return nc


# ---------------------------------------------------------------------------
# entry point
# ---------------------------------------------------------------------------

def make_in_maps(meta, W0, b0, W1, b1, W2, b2, n_cores=N_CORES):
    B = meta["B"]
    iota = np.tile(np.arange(P, dtype=np.float32), (P, 1))
    eye = np.eye(P, dtype=np.float32)
    b0b = np.tile(np.asarray(b0, np.float32), (P, 1))
    b1b = np.tile(np.asarray(b1, np.float32), (P, 1))
    b2b = np.tile(np.asarray(b2, np.float32), (P, 1))
    wmat = np.concatenate(
        [np.asarray(W0, np.float32), np.asarray(W1, np.float32),
         np.asarray(W2, np.float32)], axis=1)
    in_maps = []
    for c in range(n_cores):
        consts = np.concatenate(
            [iota, eye, b0b, b1b, b2b, meta["d1"][c], meta["d2"][c]], axis=1)
        in_maps.append({
            "x_full": meta["x_pad"],
            "xT": meta["xT"][c],
            "wmat": wmat,
            "consts": np.ascontiguousarray(consts),
            "eidx": meta["eidx16"][c],
            "erow": meta["erow"][c],
            "ew": meta["ew"][c],
        })
    return in_maps


def kernel(x, row, col, edge_weight, W0, b0, W1, b1, W2, b2):
    x = np.asarray(x, np.float32)
    row = np.asarray(row, np.int32)
    col = np.asarray(col, np.int32)
    edge_weight = np.asarray(edge_weight, np.float32)
    N = x.shape[0]

    meta = _prep(x, row, col, edge_weight)
    nc = build_program(meta)
    in_maps = make_in_maps(meta, W0, b0, W1, b1, W2, b2)
    res = run_bass_kernel_spmd(nc, in_maps, core_ids=list(range(N_CORES)))
    out = np.concatenate([r["out"] for r in res.results], axis=0)
    return np.ascontiguousarray(out[:N])


if __name__ == "__main__":
    rng = np.random.default_rng(0)
    N, C, E = 2048, 128, 8192
    x = rng.standard_normal((N, C), dtype=np.float32)
    row = rng.integers(0, N, E).astype(np.int32)
    col = rng.integers(0, N, E).astype(np.int32)
    w = rng.random(E, dtype=np.float32)
    meta = _prep(x, row, col, w)
    print("tiles/core/pass:", meta["LT"], "stages:", len(meta["stages"]))



# revision 19
# speedup vs baseline: 1.6717x; 1.6717x over previous
"""MixHop layer (gnn_message_passing) as a Trainium2 Bass/Tile SPMD kernel.

Math reformulation (A = sparse adjacency with edge weights, row=dst, col=src):
    x0 = x @ W0 + b0
    x1 = A @ (x @ W1 + b1) = (A @ x) @ W1 + d1 (x) b1      d1 = A @ 1
    x2 = A @ A @ (x @ W2 + b2) = (A @ A @ x) @ W2 + d2 (x) b2,  d2 = A @ d1
so only two sparse propagations of the raw features are needed:
    y1 = A @ x   (pass A),   y2 = A @ y1  (pass B, after all-gather of y1)

Key structure (v2):
  - Pass A is GATHER-FREE: the host pre-builds an edge-major slab
    xe[j] = w[j] * x[col[j]] (bf16), sorted by destination block and padded
    per block to 128-edge tiles.  The device streams it sequentially.
  - Destination scatter is a bf16 matmul per edge tile:
    y1^T[psum] += xe_tile^T @ mask_tile, where mask[e, r] = (row[e] == r)
    is built for a whole stage of tiles with ONE batched DVE is_equal
    (broadcast access patterns), layout [128e, 128r, T].
  - x1 is computed directly from the y1^T accumulator (evac to bf16 SBUF,
    one matmul against W1); the d1 (x) b1 bias is added with a rank-1
    (K=1) matmul into the same PSUM.  One PE transpose per block produces
    row-major y1 (bf16) for the AllGather.
  - The AllGather is chunked per stage so it overlaps pass A; the x0 loop
    is emitted between the passes to fill the AllGather tail.
  - Pass B gathers y1 rows (bf16, 256B) with gpsimd dma_gather exactly as
    before (per (block, source-chunk) tiling, int16 indices), but the
    sel = w * mask matrices are built with one batched DVE is_equal plus
    one batched Activation-engine multiply per stage.
"""

import os
import sys

import numpy as np
import ml_dtypes

for _p in ("/opt/trn_rl_repo",):
    if os.path.isdir(_p) and _p not in sys.path:
        sys.path.insert(0, _p)

import concourse.bacc as bacc
import concourse.bass as bass
import concourse.mybir as mybir
import concourse.tile as tile
from concourse.bass_utils import run_bass_kernel_spmd

F32 = mybir.dt.float32
BF16 = mybir.dt.bfloat16
I16 = mybir.dt.int16
NPBF16 = ml_dtypes.bfloat16

N_CORES = 8
P = 128          # partitions / rows per block / edges per tile
NCHUNK = 4       # source chunks for pass-B gathers (int16 index reach)
CAP = 64         # max edge tiles per stage (both passes)


# ---------------------------------------------------------------------------
# host-side preprocessing
# ---------------------------------------------------------------------------

def _to_bf16(a):
    """Fast round-to-nearest-even float32 -> bfloat16 (ml_dtypes astype is
    an order of magnitude slower)."""
    u = np.ascontiguousarray(a, dtype=np.float32).view(np.uint32)
    r = ((u >> 16) & 1) + np.uint32(0x7FFF)
    return ((u + r) >> 16).astype(np.uint16).view(NPBF16).reshape(a.shape)


def _make_stages(T_B, cap=CAP):
    """Group consecutive blocks into stages of <= cap tiles.
    Returns (first_block, n_blocks, stage_tile_offset) tuples."""
    stages = []
    b = 0
    off = 0
    B = len(T_B)
    while b < B:
        start = b
        soff = off
        tot = 0
        while b < B and (b == start or tot + T_B[b] <= cap):
            tot += int(T_B[b])
            off += int(T_B[b])
            b += 1
        stages.append((start, b - start, soff))
    return stages


def _prep(x, row, col, edge_weight, n_cores=N_CORES):
    N, C = x.shape
    E = row.shape[0]
    S = -(-N // (n_cores * P)) * P          # shard rows per core
    NP = S * n_cores                        # padded node count
    B = S // P                              # dest blocks per core
    CH = NP // NCHUNK                       # source-chunk rows
    assert CH % P == 0 and CH <= 32768

    # weighted degrees (biases of hop1/hop2 after the reformulation)
    w64 = edge_weight.astype(np.float64)
    d1 = np.bincount(row, weights=w64, minlength=NP)[:NP]
    d2 = np.bincount(row, weights=w64 * d1[col], minlength=NP)[:NP]
    d1 = d1.astype(np.float32)
    d2 = d2.astype(np.float32)

    gblk = (row // P).astype(np.int64)
    nblk = NP // P

    # ---------------- pass A: dest-block sort, gather-free slab -----------
    orderA = np.argsort(gblk, kind="stable")
    gblk_a = gblk[orderA]
    cntA = np.bincount(gblk_a, minlength=nblk)
    T_A = (-(-cntA.reshape(n_cores, B) // P)).max(axis=0).astype(np.int64)
    T_A[T_A == 0] = 1
    LT_A = int(T_A.sum())
    stagesA = _make_stages(T_A)

    baseA = np.zeros(B, dtype=np.int64)
    np.cumsum(T_A[:-1], out=baseA[1:])

    startA = np.zeros(nblk + 1, dtype=np.int64)
    np.cumsum(cntA, out=startA[1:])
    rankA = np.arange(E, dtype=np.int64) - startA[gblk_a]
    bl_a = gblk_a % B
    core_a = gblk_a // B
    posA = P * baseA[bl_a] + rankA

    LA = P * LT_A
    # xe slab: [cores, P, LT_A, C] bf16, edge j -> (tile j//P, partition j%P)
    xe = np.zeros((n_cores, LA, C), dtype=NPBF16)
    col_a = col[orderA].astype(np.int64)
    w_a = edge_weight[orderA].astype(np.float32)
    for c in range(n_cores):
        m = core_a == c
        xe[c, posA[m]] = _to_bf16(w_a[m, None] * x[col_a[m]])
    xe = np.ascontiguousarray(
        xe.reshape(n_cores, LT_A, P, C).transpose(0, 2, 1, 3))
    erow_a = np.zeros((n_cores, LA), dtype=NPBF16)
    erow_a[core_a, posA] = _to_bf16((row[orderA] % P).astype(np.float32))
    erow_a = np.ascontiguousarray(
        erow_a.reshape(n_cores, LT_A, P).transpose(0, 2, 1))

    # ---------------- pass B: (dest block, source chunk) sort -------------
    chunk = (col // CH).astype(np.int64)
    order = np.lexsort((chunk, gblk))
    gblk_s = gblk[order]
    chunk_s = chunk[order]
    col_s = col[order].astype(np.int64)
    w_s = edge_weight[order].astype(np.float32)
    rloc_s = (row[order] % P).astype(np.float32)

    grp = gblk_s * NCHUNK + chunk_s
    cnt = np.bincount(grp, minlength=nblk * NCHUNK)
    tiles = -(-cnt // P)
    T_BC = tiles.reshape(n_cores, B, NCHUNK).max(axis=0).astype(np.int64)
    empty = T_BC.sum(axis=1) == 0
    T_BC[empty, 0] = 1
    T_B = T_BC.sum(axis=1)
    LT_B = int(T_B.sum())
    stagesB = _make_stages(T_B)

    base_bc = np.zeros((B, NCHUNK), dtype=np.int64)
    pos0 = 0
    call_lens = []                                      # per stage: 4 lens
    for (b0, nb, _) in stagesB:
        lens = []
        for c in range(NCHUNK):
            n = 0
            for b in range(b0, b0 + nb):
                base_bc[b, c] = pos0
                pos0 += T_BC[b, c]
                n += T_BC[b, c]
            lens.append(int(n))
        call_lens.append(lens)
    assert pos0 == LT_B

    grp_start = np.zeros(nblk * NCHUNK + 1, dtype=np.int64)
    np.cumsum(cnt, out=grp_start[1:])
    rank = np.arange(E, dtype=np.int64) - grp_start[grp]
    b_local = gblk_s % B
    e_core = gblk_s // B
    pos = P * base_bc[b_local, chunk_s] + rank

    LB = P * LT_B
    eidx = np.zeros((n_cores, LB), dtype=np.int16)      # pad -> chunk row 0
    erow_b = np.zeros((n_cores, LB), dtype=NPBF16)
    ew_b = np.zeros((n_cores, LB), dtype=NPBF16)        # pad -> weight 0
    eidx[e_core, pos] = (col_s - chunk_s * CH).astype(np.int16)
    erow_b[e_core, pos] = _to_bf16(rloc_s)
    ew_b[e_core, pos] = _to_bf16(w_s)

    # dma_gather index wrapping: idx i -> [i % 16, i // 16]; call bases are
    # multiples of 128 so a global wrap equals per-call wraps. Replicated to
    # all 8 Q7 core groups (16-partition stripes).
    eidx16 = np.ascontiguousarray(
        np.tile(eidx.reshape(n_cores, LB // 16, 16).transpose(0, 2, 1),
                (1, 8, 1)))                             # [cores, 128, LB//16]
    erow_b = np.ascontiguousarray(
        erow_b.reshape(n_cores, LT_B, P).transpose(0, 2, 1))
    ew_b = np.ascontiguousarray(
        ew_b.reshape(n_cores, LT_B, P).transpose(0, 2, 1))

    # dense-loop inputs
    x_pad = np.zeros((NP, C), dtype=np.float32)
    x_pad[:N] = x
    xT = _to_bf16(np.ascontiguousarray(
        x_pad.reshape(n_cores, S, C).transpose(0, 2, 1)))

    d1_r = _to_bf16(d1.reshape(n_cores, S))             # [cores, S]
    d2_r = _to_bf16(d2.reshape(n_cores, S))

    return dict(N=N, C=C, S=S, NP=NP, B=B, CH=CH,
                T_A=T_A, LT_A=LT_A, stagesA=stagesA, baseA=baseA,
                xe=xe, erow_a=erow_a,
                T_BC=T_BC, T_B=T_B, LT_B=LT_B, stagesB=stagesB,
                call_lens=call_lens, base_bc=base_bc,
                eidx16=eidx16, erow_b=erow_b, ew_b=ew_b,
                xT=xT, d1=d1_r, d2=d2_r)


# ---------------------------------------------------------------------------
# device program
# ---------------------------------------------------------------------------

def build_program(meta, n_cores=N_CORES):
    C, S, NP, B, CH = meta["C"], meta["S"], meta["NP"], meta["B"], meta["CH"]
    T_A, LT_A, stagesA, baseA = (meta["T_A"], meta["LT_A"], meta["stagesA"],
                                 meta["baseA"])
    T_BC, T_B, LT_B, stagesB = (meta["T_BC"], meta["T_B"], meta["LT_B"],
                                meta["stagesB"])
    call_lens, base_bc = meta["call_lens"], meta["base_bc"]

    nc = bacc.Bacc("TRN2", target_bir_lowering=False, debug=False,
                   num_devices=n_cores, num_swdge_queues=4)

    xe_d = nc.dram_tensor("xe", [P, LT_A, C], BF16, kind="ExternalInput")
    erowa_d = nc.dram_tensor("erowa", [P, LT_A], BF16, kind="ExternalInput")
    erowb_d = nc.dram_tensor("erowb", [P, LT_B], BF16, kind="ExternalInput")
    ewb_d = nc.dram_tensor("ewb", [P, LT_B], BF16, kind="ExternalInput")
    eidx_d = nc.dram_tensor("eidx", [P, (P * LT_B) // 16], I16,
                            kind="ExternalInput")
    xT_d = nc.dram_tensor("xT", [C, S], BF16, kind="ExternalInput")
    wmat_d = nc.dram_tensor("wmat", [C, 3 * C], BF16, kind="ExternalInput")
    # [65, S]: rows 0/32/64 = d1/d2/ones (rank-1 bias matmuls; matmul
    # operand base partitions must be 0, 32 or 64)
    dmat_d = nc.dram_tensor("dmat", [65, S], BF16, kind="ExternalInput")
    # [65, 3C]: rows 0/32/64 = b0|b1|b2 (aligned with dmat rows)
    bmat_d = nc.dram_tensor("bmat", [65, 3 * C], BF16, kind="ExternalInput")
    eye_d = nc.dram_tensor("eye", [P, P], BF16, kind="ExternalInput")
    # iota_rep[p, r, t] = r
    irep_d = nc.dram_tensor("irep", [P, P, CAP], BF16, kind="ExternalInput")
    out_d = nc.dram_tensor("out", [S, 3 * C], F32, kind="ExternalOutput")

    NBA = max(nb for _, nb, _ in stagesA)
    NBB = max(nb for _, nb, _ in stagesB)

    with tile.TileContext(nc) as tc:
        with (
            tc.tile_pool(name="dram", bufs=1, space="DRAM") as dram,
            tc.tile_pool(name="cpool", bufs=1) as cpool,
            tc.tile_pool(name="fpool", bufs=3) as fpool,
            tc.tile_pool(name="mpool", bufs=2) as mpool,
            tc.tile_pool(name="ipool", bufs=3) as ipool,
            tc.tile_pool(name="vpool", bufs=3) as vpool,
            tc.tile_pool(name="ypsum", bufs=3, space="PSUM") as ypsum,
            tc.tile_pool(name="tpsum", bufs=2, space="PSUM") as tpsum,
            tc.tile_pool(name="xpsum", bufs=3, space="PSUM") as xpsum,
        ):
            # pass-B gather source, row-major [NP, C].  Collectives may only
            # have a single writer per Shared tensor, so each stage
            # all-gathers into its own Shared staging tile (y1g) and a local
            # DRAM->DRAM copy lays the rows into y1f.
            y1f = dram.tile([NP, C], BF16)
            y1s_chunks = [dram.tile([nb * P, C], BF16, name=f"y1s{si}")
                          for si, (_, nb, _) in enumerate(stagesA)]
            y1g_chunks = [dram.tile([n_cores * nb * P, C], BF16,
                                    addr_space="Shared", name=f"y1g{si}")
                          for si, (_, nb, _) in enumerate(stagesA)]

            # resident constants
            erowa_t = cpool.tile([P, LT_A], BF16, tag="erowa")
            nc.sync.dma_start(erowa_t[:], erowa_d[:])
            erowb_t = cpool.tile([P, LT_B], BF16, tag="erowb")
            nc.sync.dma_start(erowb_t[:], erowb_d[:])
            ewb_t = cpool.tile([P, LT_B], BF16, tag="ewb")
            nc.sync.dma_start(ewb_t[:], ewb_d[:])
            wmat_t = cpool.tile([C, 3 * C], BF16, tag="wmat")
            nc.sync.dma_start(wmat_t[:], wmat_d[:])
            dmat_t = cpool.tile([65, S], BF16, tag="dmat")
            nc.sync.dma_start(dmat_t[:], dmat_d[:])
            bmat_t = cpool.tile([65, 3 * C], BF16, tag="bmat")
            nc.sync.dma_start(bmat_t[:], bmat_d[:])
            eye_t = cpool.tile([P, P], BF16, tag="eye")
            nc.sync.dma_start(eye_t[:], eye_d[:])
            irep_t = cpool.tile([P, P, CAP], BF16, tag="irep")
            nc.sync.dma_start(irep_t[:], irep_d[:])

            w0_t = wmat_t[:, 0 * C:1 * C]
            w1_t = wmat_t[:, 1 * C:2 * C]
            w2_t = wmat_t[:, 2 * C:3 * C]

            def emit_bias_mm(ps, drow, col0, b):
                # ps += outer(dmat[32*drow, block b], b_{col0}): K=1 matmul
                r = 32 * drow
                nc.tensor.matmul(
                    out=ps[:],
                    lhsT=dmat_t[r:r + 1, b * P:(b + 1) * P],
                    rhs=bmat_t[r:r + 1, col0:col0 + C],
                    start=False, stop=True)

            # ---------------- pass A:  y1 = A @ x,  x1 = y1 @ W1 + d1(x)b1
            for si, (b0, nb, soff) in enumerate(stagesA):
                ts = int(T_A[b0:b0 + nb].sum())
                xe_t = fpool.tile([P, CAP, C], BF16, tag="f")
                nc.sync.dma_start(xe_t[:, :ts, :], xe_d[:, soff:soff + ts, :])
                mask_t = mpool.tile([P, P, CAP], BF16, tag="mask")
                nc.vector.tensor_tensor(
                    out=mask_t[:, :, :ts],
                    in0=erowa_t[:, soff:soff + ts].unsqueeze(1)
                        .broadcast_to([P, P, ts]),
                    in1=irep_t[:, :, :ts],
                    op=mybir.AluOpType.is_equal)

                ystg = vpool.tile([P, NBA, C], BF16, tag="ystg")
                x1st = vpool.tile([P, NBA, C], F32, tag="x1st")
                for b in range(b0, b0 + nb):
                    kk = b - b0
                    t0 = int(baseA[b] - soff)
                    nt = int(T_A[b])
                    y_ps = ypsum.tile([P, P], F32, tag="ypsum")
                    for t in range(t0, t0 + nt):
                        nc.tensor.matmul(
                            out=y_ps[:],
                            lhsT=xe_t[:, t, :],
                            rhs=mask_t[:, :, t],
                            start=(t == t0), stop=(t == t0 + nt - 1))
                    # y_ps holds y1^T for this block: [C, rows]
                    y1T_sb = vpool.tile([P, P], BF16, tag="y1T")
                    nc.vector.tensor_copy(y1T_sb[:], y_ps[:])
                    # x1 = y1 @ W1 + d1 (x) b1
                    x_ps = xpsum.tile([P, C], F32, tag="xpsum")
                    nc.tensor.matmul(out=x_ps[:], lhsT=y1T_sb[:], rhs=w1_t,
                                     start=True, stop=False)
                    emit_bias_mm(x_ps, 0, C, b)
                    nc.scalar.copy(x1st[:, kk, :], x_ps[:])
                    # row-major y1 (bf16) for the all-gather
                    yr_ps = tpsum.tile([P, P], BF16, tag="tpsum")
                    nc.tensor.transpose(yr_ps[:], y1T_sb[:], eye_t[:])
                    nc.scalar.copy(ystg[:, kk, :], yr_ps[:])

                nc.sync.dma_start(
                    y1s_chunks[si][:]
                    .rearrange("(g p) c -> p g c", p=P),
                    ystg[:, :nb, :])
                nc.sync.dma_start(
                    out_d[b0 * P:(b0 + nb) * P, C:2 * C]
                    .rearrange("(g p) c -> p g c", p=P),
                    x1st[:, :nb, :])
                nc.gpsimd.collective_compute(
                    "AllGather",
                    mybir.AluOpType.bypass,
                    replica_groups=[list(range(n_cores))],
                    ins=[y1s_chunks[si][:].opt()],
                    outs=[y1g_chunks[si][:].opt()],
                )
                # relayout y1g -> y1f rows via an SBUF bounce (DRAM->DRAM
                # DMA is not exercised anywhere else in this stack)
                y1gs = vpool.tile([P, n_cores * NBA, C], BF16, tag="y1gs")
                nc.sync.dma_start(
                    y1gs[:, :n_cores * nb, :],
                    y1g_chunks[si][:].rearrange("(g p) c -> p g c", p=P))
                for n in range(n_cores):
                    nc.sync.dma_start(
                        y1f[n * S + b0 * P:n * S + (b0 + nb) * P, :]
                        .rearrange("(g p) c -> p g c", p=P),
                        y1gs[:, n * nb:(n + 1) * nb, :])

            # ---------------- x0 = x @ W0 + b0 (fills the AG tail)
            for g0 in range(0, B, 8):
                gn = min(8, B - g0)
                xT_t = vpool.tile([C, 8 * P], BF16, tag="xT")
                nc.sync.dma_start(xT_t[:, :gn * P],
                                  xT_d[:, g0 * P:(g0 + gn) * P])
                x0st = vpool.tile([P, 8, C], F32, tag="x0st")
                for k in range(gn):
                    x_ps = xpsum.tile([P, C], F32, tag="xpsum")
                    nc.tensor.matmul(
                        out=x_ps[:], lhsT=xT_t[:, k * P:(k + 1) * P],
                        rhs=w0_t, start=True, stop=False)
                    emit_bias_mm(x_ps, 2, 0, g0 + k)
                    nc.scalar.copy(x0st[:, k, :], x_ps[:])
                nc.sync.dma_start(
                    out_d[g0 * P:(g0 + gn) * P, 0:C]
                    .rearrange("(g p) c -> p g c", p=P),
                    x0st[:, :gn, :])

            # ---------------- pass B:  y2 = A @ y1,  x2 = y2 @ W2 + d2(x)b2
            for si, (b0, nb, soff) in enumerate(stagesB):
                ts = int(T_B[b0:b0 + nb].sum())
                idx_t = ipool.tile([P, (P * CAP) // 16], I16, tag="idx")
                i16o = (P * soff) // 16
                i16n = (P * ts) // 16
                nc.sync.dma_start(idx_t[:, :i16n],
                                  eidx_d[:, i16o:i16o + i16n])
                f_t = fpool.tile([P, CAP, C], BF16, tag="f")
                rel = 0
                for c in range(NCHUNK):
                    tsc = call_lens[si][c]
                    if tsc == 0:
                        continue
                    nidx = P * tsc
                    nc.gpsimd.dma_gather(
                        out_ap=f_t[:, rel:rel + tsc, :],
                        in_ap=y1f[c * CH:(c + 1) * CH, :],
                        idxs_ap=idx_t[:, (P * rel) // 16:
                                      (P * rel) // 16 + nidx // 16],
                        num_idxs=nidx,
                        num_idxs_reg=nidx,
                        elem_size=C,
                        single_packet=False,
                        queue_num=c,
                    )
                    rel += tsc
                # sel[e, r, t] = w[e,t] * (row[e,t] == r): batched mask
                # (DVE) then in-place scale (Activation engine)
                mask_t = mpool.tile([P, P, CAP], BF16, tag="mask")
                nc.vector.tensor_tensor(
                    out=mask_t[:, :, :ts],
                    in0=erowb_t[:, soff:soff + ts].unsqueeze(1)
                        .broadcast_to([P, P, ts]),
                    in1=irep_t[:, :, :ts],
                    op=mybir.AluOpType.is_equal)
                nc.vector.tensor_tensor(
                    out=mask_t[:, :, :ts],
                    in0=mask_t[:, :, :ts],
                    in1=ewb_t[:, soff:soff + ts].unsqueeze(1)
                        .broadcast_to([P, P, ts]),
                    op=mybir.AluOpType.mult)

                x2st = vpool.tile([P, NBB, C], F32, tag="x2st")
                for b in range(b0, b0 + nb):
                    kk = b - b0
                    y_ps = ypsum.tile([P, P], F32, tag="ypsum")
                    tl = [(int(base_bc[b, c] - soff), int(T_BC[b, c]))
                          for c in range(NCHUNK) if T_BC[b, c] > 0]
                    ntile = sum(n for _, n in tl)
                    k = 0
                    for (g0, n) in tl:
                        for t in range(g0, g0 + n):
                            nc.tensor.matmul(
                                out=y_ps[:],
                                lhsT=f_t[:, t, :],
                                rhs=mask_t[:, :, t],
                                start=(k == 0), stop=(k == ntile - 1))
                            k += 1
                    y2T_sb = vpool.tile([P, P], BF16, tag="y2T")
                    nc.vector.tensor_copy(y2T_sb[:], y_ps[:])
                    x_ps = xpsum.tile([P, C], F32, tag="xpsum")
                    nc.tensor.matmul(out=x_ps[:], lhsT=y2T_sb[:], rhs=w2_t,
                                     start=True, stop=False)
                    emit_bias_mm(x_ps, 1, 2 * C, b)
                    nc.scalar.copy(x2st[:, kk, :], x_ps[:])
                nc.sync.dma_start(
                    out_d[b0 * P:(b0 + nb) * P, 2 * C:3 * C]
                    .rearrange("(g p) c -> p g c", p=P),
                    x2st[:, :nb, :])

    nc.compile()
    return nc


# ---------------------------------------------------------------------------
# entry point
# ---------------------------------------------------------------------------

def make_in_maps(meta, W0, b0, W1, b1, W2, b2, n_cores=N_CORES):
    C = meta["C"]
    eye = np.eye(P, dtype=NPBF16)
    irep = np.ascontiguousarray(np.broadcast_to(
        np.arange(P, dtype=NPBF16)[None, :, None], (P, P, CAP)))
    wmat = np.concatenate(
        [np.asarray(W0, np.float32), np.asarray(W1, np.float32),
         np.asarray(W2, np.float32)], axis=1).astype(NPBF16)
    brow = np.concatenate(
        [np.asarray(b0, np.float32), np.asarray(b1, np.float32),
         np.asarray(b2, np.float32)]).astype(NPBF16)
    bmat = np.zeros((65, 3 * C), dtype=NPBF16)
    bmat[0] = bmat[32] = bmat[64] = brow
    in_maps = []
    S = meta["d1"].shape[1]
    for c in range(n_cores):
        dmat = np.zeros((65, S), dtype=NPBF16)
        dmat[0] = meta["d1"][c]
        dmat[32] = meta["d2"][c]
        dmat[64] = 1.0
        in_maps.append({
            "xe": meta["xe"][c],
            "erowa": meta["erow_a"][c],
            "erowb": meta["erow_b"][c],
            "ewb": meta["ew_b"][c],
            "eidx": meta["eidx16"][c],
            "xT": meta["xT"][c],
            "wmat": wmat,
            "dmat": np.ascontiguousarray(dmat),
            "bmat": np.ascontiguousarray(bmat),
            "eye": eye,
            "irep": irep,
        })
    return in_maps


def kernel(x, row, col, edge_weight, W0, b0, W1, b1, W2, b2):
    x = np.asarray(x, np.float32)
    row = np.asarray(row, np.int32)
    col = np.asarray(col, np.int32)
    edge_weight = np.asarray(edge_weight, np.float32)
    N = x.shape[0]

    meta = _prep(x, row, col, edge_weight)
    nc = build_program(meta)
    in_maps = make_in_maps(meta, W0, b0, W1, b1, W2, b2)
    res = run_bass_kernel_spmd(nc, in_maps, core_ids=list(range(N_CORES)))
    out = np.concatenate([r["out"] for r in res.results], axis=0)
    return np.ascontiguousarray(out[:N])


if __name__ == "__main__":
    rng = np.random.default_rng(0)
    N, C, E = 2048, 128, 8192
    x = rng.standard_normal((N, C), dtype=np.float32)
    row = rng.integers(0, N, E).astype(np.int32)
    col = rng.integers(0, N, E).astype(np.int32)
    w = rng.random(E, dtype=np.float32)
    meta = _prep(x, row, col, w)
    print("pass A tiles:", meta["LT_A"], "stages:", len(meta["stagesA"]))
    print("pass B tiles:", meta["LT_B"], "stages:", len(meta["stagesB"]))
